# revision 15
# baseline (speedup 1.0000x reference)
"""Multi-head attention (B=2, H=8, S=2048, hd=16) on 8 Trainium2 NeuronCores.

Sharding: 16 (batch, head) groups -> 2 heads per core (cores 0-3: batch 0,
cores 4-7: batch 1).  Each core gets transposed embeddings, a key-compacted
copy (keys with source-mask 0 dropped; padded to NK with -1000 mask columns),
and the 32 projection-weight columns for its two heads.

Score matmuls run in float32r (1 cycle/row on the PE vs 4 for fp32) with
fp32-level accuracy recovered via split-precision row packing: K and Q are
each split into bf16-high + fp32-residual parts (Kh+Kl, Qh+Ql) and the four
cross products are packed into one 128-row contraction
  rows  0:16  Kh x Qh        rows 32:48  Kl x Qh(dup)
  rows 64:80  Kh(dup) x Ql   rows 96:112 Kl(dup) x Ql(dup)
  row 16: mask x ones        row 17: ones x (-rowmax)      (gaps zeroed)
Extra contraction rows are free (matmul cost is N output columns only), and
bf16-grid values pass through the PE's f32r truncation unchanged, so the sum
reconstructs the exact fp32 product.

Row-max comes from a cheap pass over the first NA=512 compacted keys (f32r,
bf16-grade): a lower-bound max keeps exp finite unless the excluded keys beat
the subsample by >88 (logistic tail, <1% of rows); those rows come back as
inf/NaN/zero and are recomputed exactly on the host.

ctx = P^T @ [V | 1] accumulates in PSUM with f32r operands; the ones column
gives the softmax denominator l; 1/l is computed on a [128,16] reshape (not
the serial [1,2048] row) and applied via DRAM-broadcast + DVE multiply.
Output per core is [32, 2048] (dim-major); the host scatters back into the
interleaved head layout.
"""

import numpy as np

S = 2048
E = 128
HD = 16
NQB = S // 128       # 16 query blocks
NEG = -1000.0
NA = 512             # keys sampled for the row-max pass

_PROGS = {}


def _build_program(NKB):
    import concourse.mybir as mybir
    from concourse import bacc
    from concourse.tile import TileContext

    NK = 128 * NKB

    fp32 = mybir.dt.float32
    f32r = mybir.dt.float32r
    bf16 = mybir.dt.bfloat16
    AF = mybir.ActivationFunctionType
    ALU = mybir.AluOpType
    AX = mybir.AxisListType

    nc = bacc.Bacc()

    xT = nc.declare_dram_parameter("xT", [E, S], fp32, isOutput=False)
    xkT = nc.declare_dram_parameter("xkT", [E, NK], fp32, isOutput=False)
    # weight columns padded to 48: head0 dims at 0:16, head1 dims at 32:48
    wq = nc.declare_dram_parameter("wq", [E, 48], fp32, isOutput=False)
    wk = nc.declare_dram_parameter("wk", [E, 48], fp32, isOutput=False)
    wv = nc.declare_dram_parameter("wv", [E, 48], fp32, isOutput=False)
    maskrow = nc.declare_dram_parameter("maskrow", [1, NK], f32r, isOutput=False)
    onesrow = nc.declare_dram_parameter("onesrow", [1, S], f32r, isOutput=False)
    zrow = nc.declare_dram_parameter("zrow", [1, S], f32r, isOutput=False)
    ident = nc.declare_dram_parameter("ident", [E, E], fp32, isOutput=False)
    out_d = nc.declare_dram_parameter("out", [2 * HD, S], fp32, isOutput=True)
    ldram = nc.dram_tensor("ldram", [2, S], fp32)

    with TileContext(nc) as tc:
        with (
            tc.tile_pool(name="consts", bufs=1) as cpool,
            tc.tile_pool(name="work", bufs=1) as wpool,
            tc.tile_pool(name="ptp", bufs=3) as ptpool,
            tc.tile_pool(name="stp", bufs=2, space="PSUM") as stpool,
            tc.tile_pool(name="ap", bufs=2, space="PSUM") as apool,
            tc.tile_pool(name="ctxp", bufs=2, space="PSUM") as ctxpool,
        ):
            # ---------------- input loads first (sync-queue order) ----------
            xT_sb = cpool.tile([E, S], fp32, name="xT_sb")
            wq_sb = cpool.tile([E, 48], fp32, name="wq_sb")
            wk_sb = cpool.tile([E, 48], fp32, name="wk_sb")
            wv_sb = cpool.tile([E, 48], fp32, name="wv_sb")
            xkT_sb = cpool.tile([E, NK], fp32, name="xkT_sb")
            ident_sb = cpool.tile([E, E], fp32, name="ident_sb")
            nc.sync.dma_start(out=xT_sb[:, 0:1024], in_=xT[:, 0:1024])
            nc.sync.dma_start(out=wq_sb[:, :], in_=wq[:, :])
            nc.sync.dma_start(out=xkT_sb[:, :], in_=xkT[:, :])
            nc.sync.dma_start(out=wk_sb[:, :], in_=wk[:, :])
            nc.sync.dma_start(out=xT_sb[:, 1024:2048], in_=xT[:, 1024:2048])
            nc.sync.dma_start(out=wv_sb[:, :], in_=wv[:, :])
            nc.sync.dma_start(out=ident_sb[:, :], in_=ident[:, :])

            # ---------------- persistent work tensors ----------------
            qt = [wpool.tile([128, S], f32r, name=f"qt{h}") for h in range(2)]
            kt = [wpool.tile([128, NK], f32r, name=f"kt{h}") for h in range(2)]
            qhb48 = wpool.tile([48, S], bf16, name="qhb48")
            khb48 = wpool.tile([48, NK], bf16, name="khb48")
            negid = wpool.tile([48, 48], bf16, name="negid")
            vv = [wpool.tile([128, NKB, HD + 1], f32r, name=f"vv{h}") for h in range(2)]
            negp = [wpool.tile([128, NQB], fp32, name=f"negp{h}") for h in range(2)]
            nT8 = [
                [wpool.tile([NQB // 2, 128], f32r, name=f"nT8_{h}{hf}") for hf in range(2)]
                for h in range(2)
            ]
            ctxl = wpool.tile([49, S], fp32, name="ctxl")
            lsq = wpool.tile([128, 2 * HD], fp32, name="lsq")
            lisq = wpool.tile([128, 2 * HD], fp32, name="lisq")
            lbc = wpool.tile([48, S], fp32, name="lbc")
            out_sb = wpool.tile([64, S], fp32, name="out_sb")

            # special rows (zero fills are issued later, after the first
            # projection loads, to keep the DMA queues free for the critical
            # input loads)
            for h in range(2):
                nc.sync.dma_start(out=qt[h][16:17, :], in_=onesrow[:, :])
                nc.sync.dma_start(out=kt[h][16:17, :], in_=maskrow[:, :])
                nc.sync.dma_start(out=kt[h][17:18, :], in_=onesrow[:, 0:NK])
                nc.sync.dma_start(
                    out=vv[h][:, :, HD : HD + 1],
                    in_=onesrow[0:1, 0:NKB].to_broadcast([128, NKB]),
                )

            nc.scalar.mul(negid[:, :], ident_sb[0:48, 0:48], -1.0)

            def zero_fills():
                # zero the unused contraction rows (both sides: 0 * 0 avoids
                # NaN from stale SBUF).  Rows 32:48/64:80/96:112 inside
                # [18:128) are re-written by the split producers; WAW deps
                # keep the order right.
                for h in range(2):
                    nc.sync.dma_start(
                        out=qt[h][18:128, :], in_=zrow[0:1, 0:S].to_broadcast([110, S])
                    )
                    nc.sync.dma_start(
                        out=kt[h][18:128, :], in_=zrow[0:1, 0:NK].to_broadcast([110, NK])
                    )

            # ---------------- projections + splits ----------------
            # Q: 1/sqrt(hd) folded into wq host-side.  Per 1024-col half:
            def q_proj(half):
                cs = slice(1024 * half, 1024 * (half + 1))
                qt_ps = stpool.tile([48, 1024], fp32, name="qt_ps", tag="st")
                for c in range(2):
                    nc.tensor.matmul(
                        qt_ps[:, 512 * c : 512 * (c + 1)],
                        lhsT=wq_sb[:, :],
                        rhs=xT_sb[:, 1024 * half + 512 * c : 1024 * half + 512 * (c + 1)],
                        start=True,
                        stop=False,
                    )
                nc.scalar.copy(qhb48[:, cs], qt_ps[:, :])        # bf16 round, both heads
                for c in range(2):                                    # PSUM -= Qh -> Ql
                    nc.tensor.matmul(
                        qt_ps[:, 512 * c : 512 * (c + 1)],
                        lhsT=negid[:, :],
                        rhs=qhb48[:, 1024 * half + 512 * c : 1024 * half + 512 * (c + 1)],
                        start=False,
                        stop=True,
                    )
                for h in range(2):
                    nc.gpsimd.tensor_copy(
                        out=qt[h][0:16, cs], in_=qhb48[32 * h : 32 * h + 16, cs]
                    )
                    nc.scalar.copy(qt[h][64:80, cs], qt_ps[32 * h : 32 * h + 16, :])
                    nc.sync.dma_start(out=qt[h][32:48, cs], in_=qt[h][0:16, cs])
                    nc.sync.dma_start(out=qt[h][96:112, cs], in_=qt[h][64:80, cs])

            def k_proj(o, n):
                cs = slice(o, o + n)
                kt_ps = stpool.tile([48, 512], fp32, name="kt_ps", tag="st")
                nc.tensor.matmul(
                    kt_ps[:, 0:n], lhsT=wk_sb[:, :], rhs=xkT_sb[:, cs], start=True, stop=False
                )
                nc.scalar.copy(khb48[:, cs], kt_ps[:, 0:n])           # bf16 round, both heads
                nc.tensor.matmul(                                     # PSUM -= Kh -> Kl
                    kt_ps[:, 0:n], lhsT=negid[:, :], rhs=khb48[:, cs], start=False, stop=True
                )
                for h in range(2):
                    nc.gpsimd.tensor_copy(
                        out=kt[h][0:16, cs], in_=khb48[32 * h : 32 * h + 16, cs]
                    )
                    nc.scalar.copy(kt[h][32:48, cs], kt_ps[32 * h : 32 * h + 16, 0:n])

            def k_dups():
                for h in range(2):
                    nc.sync.dma_start(out=kt[h][64:80, :], in_=kt[h][0:16, :])
                    nc.sync.dma_start(out=kt[h][96:112, :], in_=kt[h][32:48, :])

            def v_iter(kb):
                v_ps = apool.tile([128, 48], fp32, name="v_ps", tag="a")
                nc.tensor.matmul(
                    v_ps[:, :],
                    lhsT=xkT_sb[:, 128 * kb : 128 * (kb + 1)],
                    rhs=wv_sb[:, :],
                    start=True,
                    stop=True,
                )
                nc.vector.tensor_copy(out=vv[0][:, kb, 0:HD], in_=v_ps[:, 0:16])
                nc.vector.tensor_copy(out=vv[1][:, kb, 0:HD], in_=v_ps[:, 32:48])

            # ---------------- pass A: subsampled row-max ----------------
            def a_iter(h, qb):
                sc = apool.tile([128, NA], fp32, name="sc", tag="a")
                nc.tensor.matmul(
                    sc[:, :],
                    lhsT=qt[h][0:17, 128 * qb : 128 * (qb + 1)],
                    rhs=kt[h][0:17, 0:NA],
                    start=True,
                    stop=True,
                )
                nc.vector.tensor_reduce(
                    negp[h][:, qb : qb + 1], sc[:, :], axis=AX.X, op=ALU.max, negate=True
                )

            def negm_half(h, hf):
                nq = NQB // 2
                ntp = apool.tile([nq, 128], fp32, name="ntp", tag="a")
                nc.tensor.transpose(
                    ntp[:, :], negp[h][:, nq * hf : nq * (hf + 1)], ident_sb[:, :]
                )
                nc.vector.tensor_copy(out=nT8[h][hf][:, :], in_=ntp[:, :])
                nc.sync.dma_start(
                    out=qt[h][17:18, 1024 * hf : 1024 * (hf + 1)].rearrange(
                        "a (b f) -> a b f", b=nq
                    ),
                    in_=nT8[h][hf][:, :],
                )

            # ---------------- pass B + ctx ----------------
            def b_iter(h, qh, kb, ctxc):
                st = stpool.tile([128, 1024], fp32, name="st", tag="st")
                lhs = kt[h][:, 128 * kb : 128 * (kb + 1)]
                for c in range(2):
                    nc.tensor.matmul(
                        st[:, 512 * c : 512 * (c + 1)],
                        lhsT=lhs,
                        rhs=qt[h][:, 1024 * qh + 512 * c : 1024 * qh + 512 * (c + 1)],
                        start=True,
                        stop=True,
                    )
                pt = ptpool.tile([128, 1024], f32r, name="pt", tag="pt")
                nc.scalar.activation(pt[:, :], st[:, :], AF.Exp)
                for c in range(2):
                    nc.tensor.matmul(
                        ctxc[c][0:17, :],
                        lhsT=vv[h][:, kb, :],
                        rhs=pt[:, 512 * c : 512 * (c + 1)],
                        start=(kb == 0),
                        stop=(kb == NKB - 1),
                    )

            def evac(h, qh, ctxc, last=False):
                eng = nc.vector if last else nc.scalar
                for c in range(2):
                    dst = ctxl[
                        32 * h : 32 * h + 17,
                        1024 * qh + 512 * c : 1024 * qh + 512 * (c + 1),
                    ]
                    if last:
                        nc.vector.tensor_copy(out=dst, in_=ctxc[c][0:17, :])
                    else:
                        nc.scalar.copy(dst, ctxc[c][0:17, :])

            def b_half(h, qh):
                return [
                    ctxpool.tile([17, 512], fp32, name=f"ctx{c}", tag="ctx")
                    for c in range(2)
                ]

            def finals(h, qh, ctxc=None):
                # l half-row -> [128,8] square, parallel reciprocal, back out
                q0 = 1024 * qh
                co = 16 * h + 8 * qh
                if False:
                    pass
                else:
                    nc.sync.dma_start(
                        out=lsq[:, co : co + 8],
                        in_=ctxl[32 * h + 16 : 32 * h + 17, q0 : q0 + 1024].rearrange(
                            "a (b f) -> a b f", b=128
                        ),
                    )
                nc.vector.reciprocal(lisq[:, co : co + 8], lsq[:, co : co + 8])
                nc.sync.dma_start(
                    out=ldram[h : h + 1, q0 : q0 + 1024].rearrange(
                        "a (b f) -> a b f", b=128
                    ),
                    in_=lisq[:, co : co + 8],
                )
                nc.sync.dma_start(
                    out=lbc[32 * h : 32 * h + 16, q0 : q0 + 1024],
                    in_=ldram[h : h + 1, q0 : q0 + 1024].to_broadcast([HD, 1024]),
                )
                nc.vector.tensor_tensor(
                    out=out_sb[32 * h : 32 * h + 16, q0 : q0 + 1024],
                    in0=ctxl[32 * h : 32 * h + 16, q0 : q0 + 1024],
                    in1=lbc[32 * h : 32 * h + 16, q0 : q0 + 1024],
                    op=ALU.mult,
                )
                nc.sync.dma_start(
                    out=out_d[16 * h : 16 * h + 16, q0 : q0 + 1024],
                    in_=out_sb[32 * h : 32 * h + 16, q0 : q0 + 1024],
                )

            # ---------------- schedule ----------------
            zero_fills()
            q_proj(0)
            k_proj(0, 512)
            # A(h0) first half interleaved with remaining projections (K first:
            # pass A and the kt dups depend on K, not on Q half 1)
            rest = [("k", o, min(512, NK - o)) for o in range(512, NK, 512)] + [
                ("q", 1, 0)
            ]
            for qb in range(NQB // 2):
                a_iter(0, qb)
                if rest:
                    kind, a1, a2 = rest.pop(0)
                    q_proj(a1) if kind == "q" else k_proj(a1, a2)
                if qb == 4:
                    k_dups()
            v_iter(0)
            v_iter(1)
            negm_half(0, 0)

            # B(h0,qh0) | A(h0) second half + V tail
            ctxc = b_half(0, 0)
            for kb in range(NKB):
                b_iter(0, 0, kb, ctxc)
                if kb < NQB // 2:
                    a_iter(0, NQB // 2 + kb)
                if kb + 2 < NKB:
                    v_iter(kb + 2)
            negm_half(0, 1)
            evac(0, 0, ctxc)

            # B(h0,qh1) | A(h1) first half
            ctxc = b_half(0, 1)
            for kb in range(NKB):
                b_iter(0, 1, kb, ctxc)
                if kb < NQB // 2:
                    a_iter(1, kb)
            negm_half(1, 0)
            evac(0, 1, ctxc)
            finals(0, 0)

            # B(h1,qh0) | A(h1) second half
            ctxc = b_half(1, 0)
            for kb in range(NKB):
                b_iter(1, 0, kb, ctxc)
                if kb < NQB // 2:
                    a_iter(1, NQB // 2 + kb)
            negm_half(1, 1)
            evac(1, 0, ctxc)
            finals(0, 1)

            # B(h1,qh1)
            ctxc = b_half(1, 1)
            for kb in range(NKB):
                b_iter(1, 1, kb, ctxc)
            finals(1, 0)
            evac(1, 1, ctxc, last=True)
            finals(1, 1)

    nc.finalize()
    return nc


def _prep_core_inputs(x, msk_add_full, w_query, w_key, w_value):
    """Build the 8 per-core input maps from full inputs.  Returns (maps, NKB)."""
    B = x.shape[0]
    onesrow = np.ones((1, S), dtype=np.float32)
    zrow = np.zeros((1, S), dtype=np.float32)
    identm = np.eye(E, dtype=np.float32)

    keeps = [np.flatnonzero(msk_add_full[b] == 0.0) for b in range(B)]
    max_nk = max(len(k) for k in keeps)
    assert max_nk >= NA, "row-max subsample needs >= NA valid keys"
    NKB = -(-max_nk // 128)  # ceil to 128
    NK = 128 * NKB

    per_batch = []
    for b in range(B):
        keep = keeps[b]
        nk = len(keep)
        xk = np.zeros((NK, E), dtype=np.float32)
        xk[:nk] = x[b][keep]
        maskrow = np.full((1, NK), NEG, dtype=np.float32)
        maskrow[0, :nk] = 0.0
        xTb = np.ascontiguousarray(x[b].T)
        xkTb = np.ascontiguousarray(xk.T)
        per_batch.append((xTb, xkTb, maskrow))

    in_maps = []
    for c in range(8):
        b = c // 4
        h0 = 2 * (c % 4)
        xTb, xkTb, maskrow = per_batch[b]

        def _pad48(w, scale=1.0):
            wc = np.zeros((E, 48), dtype=np.float32)
            wc[:, 0:16] = w[:, h0::8] * scale
            wc[:, 32:48] = w[:, h0 + 1 :: 8] * scale
            return wc

        in_maps.append(
            {
                "xT": xTb,
                "xkT": xkTb,
                "wq": _pad48(w_query, 0.25),  # 1/sqrt(hd) folded in (exact)
                "wk": _pad48(w_key),
                "wv": _pad48(w_value),
                "maskrow": maskrow,
                "onesrow": onesrow,
                "zrow": zrow,
                "ident": identm,
            }
        )
    return in_maps, NKB


def kernel(
    input_embeddings,
    token_attention_masks_source,
    token_attention_masks_target,
    masked,
    w_query,
    w_key,
    w_value,
):
    x = np.asarray(input_embeddings, dtype=np.float32)
    msk = np.asarray(token_attention_masks_source)
    wq_f = np.asarray(w_query, dtype=np.float32)
    wk_f = np.asarray(w_key, dtype=np.float32)
    wv_f = np.asarray(w_value, dtype=np.float32)
    assert int(np.asarray(masked)) == 0, "only the encoder (masked=0) path is supported"
    B = x.shape[0]
    assert x.shape == (2, S, E)

    msk_add = np.where(msk == 0, np.float32(NEG), np.float32(0.0))
    in_maps, NKB = _prep_core_inputs(x, msk_add, wq_f, wk_f, wv_f)

    if NKB not in _PROGS:
        _PROGS[NKB] = _build_program(NKB)
    nc = _PROGS[NKB]
    global _PROG
    _PROG = nc

    from concourse.bass_utils import run_bass_kernel_spmd

    res = run_bass_kernel_spmd(nc, in_maps, list(range(8)))

    out = np.empty((B, S, E), dtype=np.float32)
    for c in range(8):
        b = c // 4
        h0 = 2 * (c % 4)
        o = res.results[c]["out"]  # [32, 2048]
        out[b][:, h0::8] = o[0:16, :].T
        out[b][:, h0 + 1 :: 8] = o[16:32, :].T

    # The device row-max is a lower bound from a 512-key subsample; rows where
    # the true max exceeds it by >~88 overflow exp to inf (-> inf or NaN or,
    # when only the denominator overflows, an exact-zero vector).  Those rows
    # are deterministic and rare (<1%); recompute them exactly on host.
    for b in range(B):
        for h in range(8):
            hv = out[b][:, h::8]  # [S, 16]
            bad = ~np.isfinite(hv).all(axis=1) | (hv == 0.0).all(axis=1)
            if not bad.any():
                continue
            rows = np.flatnonzero(bad)
            xb = x[b].astype(np.float64)
            qh = (xb[rows] @ wq_f[:, h::8].astype(np.float64)) * 0.25
            kh = xb @ wk_f[:, h::8].astype(np.float64)
            vh = xb @ wv_f[:, h::8].astype(np.float64)
            sc = qh @ kh.T + msk_add[b][None, :].astype(np.float64)
            sc -= sc.max(axis=1, keepdims=True)
            p = np.exp(sc)
            p /= p.sum(axis=1, keepdims=True)
            out[b][rows, h::8] = (p @ vh).astype(np.float32)
    return out


_PROG = None


# revision 17
# speedup vs baseline: 1.1408x; 1.1408x over previous
"""Multi-head attention (B=2, H=8, S=2048, hd=16) on 8 Trainium2 NeuronCores.

Sharding: 16 (batch, head) groups -> 2 heads per core (cores 0-3: batch 0,
cores 4-7: batch 1).  Each core gets transposed embeddings, a key-compacted
copy (keys with source-mask 0 dropped; padded to NK with -1000 mask columns),
and the 32 projection-weight columns for its two heads.

Score matmuls run in float32r (1 cycle/row on the PE vs 4 for fp32) with
fp32-level accuracy recovered via split-precision row packing: K and Q are
each split into bf16-high + fp32-residual parts (Kh+Kl, Qh+Ql) and the four
cross products are packed into one 128-row contraction
  rows  0:16  Kh x Qh        rows 32:48  Kl x Qh(dup)
  rows 64:80  Kh(dup) x Ql   rows 96:112 Kl(dup) x Ql(dup)
  row 16: mask x ones        row 17: ones x (-rowmax)      (gaps zeroed)
Extra contraction rows are free (matmul cost is N output columns only), and
bf16-grid values pass through the PE's f32r truncation unchanged, so the sum
reconstructs the exact fp32 product.

Row-max comes from a cheap pass over the first NA=512 compacted keys (f32r,
bf16-grade): a lower-bound max keeps exp finite unless the excluded keys beat
the subsample by >88 (logistic tail, <1% of rows); those rows come back as
inf/NaN/zero and are recomputed exactly on the host.

ctx = P^T @ [V | 1] accumulates in PSUM with f32r operands; the ones column
gives the softmax denominator l; 1/l is computed on a [128,16] reshape (not
the serial [1,2048] row) and applied via DRAM-broadcast + DVE multiply.
Output per core is [32, 2048] (dim-major); the host scatters back into the
interleaved head layout.
"""

import numpy as np

S = 2048
E = 128
HD = 16
NQB = S // 128       # 16 query blocks
NEG = -1000.0
NA = 512             # keys sampled for the row-max pass

_PROGS = {}


def _build_program(NKB):
    import concourse.mybir as mybir
    from concourse import bacc
    from concourse.tile import TileContext

    NK = 128 * NKB

    fp32 = mybir.dt.float32
    f32r = mybir.dt.float32r
    bf16 = mybir.dt.bfloat16
    AF = mybir.ActivationFunctionType
    ALU = mybir.AluOpType
    AX = mybir.AxisListType

    nc = bacc.Bacc()

    xT = nc.declare_dram_parameter("xT", [E, S], fp32, isOutput=False)
    xkT = nc.declare_dram_parameter("xkT", [E, NK], fp32, isOutput=False)
    # weight columns padded to 48: head0 dims at 0:16, head1 dims at 32:48
    wq = nc.declare_dram_parameter("wq", [E, 48], fp32, isOutput=False)
    wk = nc.declare_dram_parameter("wk", [E, 48], fp32, isOutput=False)
    wv = nc.declare_dram_parameter("wv", [E, 48], fp32, isOutput=False)
    maskrow = nc.declare_dram_parameter("maskrow", [1, NK], f32r, isOutput=False)
    onesrow = nc.declare_dram_parameter("onesrow", [1, S], f32r, isOutput=False)
    zrow = nc.declare_dram_parameter("zrow", [1, S], f32r, isOutput=False)
    ident = nc.declare_dram_parameter("ident", [E, E], fp32, isOutput=False)
    out_d = nc.declare_dram_parameter("out", [2 * HD, S], fp32, isOutput=True)
    ldram = nc.dram_tensor("ldram", [2, S], fp32)

    with TileContext(nc) as tc:
        with (
            tc.tile_pool(name="consts", bufs=1) as cpool,
            tc.tile_pool(name="work", bufs=1) as wpool,
            tc.tile_pool(name="ptp", bufs=3) as ptpool,
            tc.tile_pool(name="stp", bufs=2, space="PSUM") as stpool,
            tc.tile_pool(name="ap", bufs=2, space="PSUM") as apool,
            tc.tile_pool(name="ctxp", bufs=2, space="PSUM") as ctxpool,
        ):
            # ---------------- input loads first (sync-queue order) ----------
            xT_sb = cpool.tile([E, S], fp32, name="xT_sb")
            wq_sb = cpool.tile([E, 48], fp32, name="wq_sb")
            wk_sb = cpool.tile([E, 48], fp32, name="wk_sb")
            wv_sb = cpool.tile([E, 48], fp32, name="wv_sb")
            xkT_sb = cpool.tile([E, NK], fp32, name="xkT_sb")
            ident_sb = cpool.tile([E, E], fp32, name="ident_sb")
            nc.sync.dma_start(out=xT_sb[:, 0:1024], in_=xT[:, 0:1024])
            nc.sync.dma_start(out=wq_sb[:, :], in_=wq[:, :])
            nc.sync.dma_start(out=xkT_sb[:, :], in_=xkT[:, :])
            nc.sync.dma_start(out=wk_sb[:, :], in_=wk[:, :])
            nc.sync.dma_start(out=xT_sb[:, 1024:2048], in_=xT[:, 1024:2048])
            nc.sync.dma_start(out=wv_sb[:, :], in_=wv[:, :])
            nc.sync.dma_start(out=ident_sb[:, :], in_=ident[:, :])

            # ---------------- persistent work tensors ----------------
            qt = [wpool.tile([128, S], f32r, name=f"qt{h}") for h in range(2)]
            kt = [wpool.tile([128, NK], f32r, name=f"kt{h}") for h in range(2)]
            qhb48 = wpool.tile([48, S], bf16, name="qhb48")
            khb48 = wpool.tile([48, NK], bf16, name="khb48")
            negid = wpool.tile([48, 48], bf16, name="negid")
            vv = [wpool.tile([128, NKB, HD + 1], f32r, name=f"vv{h}") for h in range(2)]
            negp = [wpool.tile([128, NQB], fp32, name=f"negp{h}") for h in range(2)]
            nT8 = [
                [wpool.tile([NQB // 2, 128], f32r, name=f"nT8_{h}{hf}") for hf in range(2)]
                for h in range(2)
            ]
            ctxl = wpool.tile([49, S], fp32, name="ctxl")
            lsq = wpool.tile([128, 2 * HD], fp32, name="lsq")
            lisq = wpool.tile([128, 2 * HD], fp32, name="lisq")
            lbc = wpool.tile([48, S], fp32, name="lbc")
            out_sb = wpool.tile([64, S], fp32, name="out_sb")

            # special rows (zero fills are issued later, after the first
            # projection loads, to keep the DMA queues free for the critical
            # input loads)
            for h in range(2):
                nc.sync.dma_start(out=qt[h][16:17, :], in_=onesrow[:, :])
                nc.sync.dma_start(out=kt[h][16:17, :], in_=maskrow[:, :])
                nc.sync.dma_start(out=kt[h][17:18, :], in_=onesrow[:, 0:NK])
                nc.sync.dma_start(
                    out=vv[h][:, :, HD : HD + 1],
                    in_=onesrow[0:1, 0:NKB].to_broadcast([128, NKB]),
                )

            nc.scalar.mul(negid[:, :], ident_sb[0:48, 0:48], -1.0)

            def zero_fills():
                # zero the unused contraction rows (both sides: 0 * 0 avoids
                # NaN from stale SBUF).  Rows 32:48/64:80/96:112 inside
                # [18:128) are re-written by the split producers; WAW deps
                # keep the order right.
                for h in range(2):
                    nc.sync.dma_start(
                        out=qt[h][18:128, :], in_=zrow[0:1, 0:S].to_broadcast([110, S])
                    )
                    nc.sync.dma_start(
                        out=kt[h][18:128, :], in_=zrow[0:1, 0:NK].to_broadcast([110, NK])
                    )

            # ---------------- projections + splits ----------------
            # Q: 1/sqrt(hd) folded into wq host-side.  Per 1024-col half:
            def q_proj(half):
                cs = slice(1024 * half, 1024 * (half + 1))
                qt_ps = stpool.tile([48, 1024], fp32, name="qt_ps", tag="st")
                for c in range(2):
                    nc.tensor.matmul(
                        qt_ps[:, 512 * c : 512 * (c + 1)],
                        lhsT=wq_sb[:, :],
                        rhs=xT_sb[:, 1024 * half + 512 * c : 1024 * half + 512 * (c + 1)],
                        start=True,
                        stop=False,
                    )
                nc.scalar.copy(qhb48[:, cs], qt_ps[:, :])        # bf16 round, both heads
                for c in range(2):                                    # PSUM -= Qh -> Ql
                    nc.tensor.matmul(
                        qt_ps[:, 512 * c : 512 * (c + 1)],
                        lhsT=negid[:, :],
                        rhs=qhb48[:, 1024 * half + 512 * c : 1024 * half + 512 * (c + 1)],
                        start=False,
                        stop=True,
                    )
                for h in range(2):
                    nc.vector.tensor_copy(
                        out=qt[h][0:16, cs], in_=qhb48[32 * h : 32 * h + 16, cs]
                    )
                    nc.vector.tensor_copy(
                        out=qt[h][64:80, cs], in_=qt_ps[32 * h : 32 * h + 16, :]
                    )
                    nc.sync.dma_start(out=qt[h][32:48, cs], in_=qt[h][0:16, cs])
                    nc.sync.dma_start(out=qt[h][96:112, cs], in_=qt[h][64:80, cs])

            def k_proj(o, n):
                cs = slice(o, o + n)
                kt_ps = stpool.tile([48, 512], fp32, name="kt_ps", tag="st")
                nc.tensor.matmul(
                    kt_ps[:, 0:n], lhsT=wk_sb[:, :], rhs=xkT_sb[:, cs], start=True, stop=False
                )
                nc.scalar.copy(khb48[:, cs], kt_ps[:, 0:n])           # bf16 round, both heads
                nc.tensor.matmul(                                     # PSUM -= Kh -> Kl
                    kt_ps[:, 0:n], lhsT=negid[:, :], rhs=khb48[:, cs], start=False, stop=True
                )
                for h in range(2):
                    nc.vector.tensor_copy(
                        out=kt[h][0:16, cs], in_=khb48[32 * h : 32 * h + 16, cs]
                    )
                    nc.vector.tensor_copy(
                        out=kt[h][32:48, cs], in_=kt_ps[32 * h : 32 * h + 16, 0:n]
                    )

            def k_dups():
                for h in range(2):
                    nc.sync.dma_start(out=kt[h][64:80, :], in_=kt[h][0:16, :])
                    nc.sync.dma_start(out=kt[h][96:112, :], in_=kt[h][32:48, :])

            def v_iter(kb):
                v_ps = apool.tile([128, 48], fp32, name="v_ps", tag="a")
                nc.tensor.matmul(
                    v_ps[:, :],
                    lhsT=xkT_sb[:, 128 * kb : 128 * (kb + 1)],
                    rhs=wv_sb[:, :],
                    start=True,
                    stop=True,
                )
                nc.vector.tensor_copy(out=vv[0][:, kb, 0:HD], in_=v_ps[:, 0:16])
                nc.vector.tensor_copy(out=vv[1][:, kb, 0:HD], in_=v_ps[:, 32:48])

            # ---------------- pass A: subsampled row-max ----------------
            def a_iter(h, qb):
                sc = apool.tile([128, NA], fp32, name="sc", tag="a")
                nc.tensor.matmul(
                    sc[:, :],
                    lhsT=qt[h][0:17, 128 * qb : 128 * (qb + 1)],
                    rhs=kt[h][0:17, 0:NA],
                    start=True,
                    stop=True,
                )
                nc.vector.tensor_reduce(
                    negp[h][:, qb : qb + 1], sc[:, :], axis=AX.X, op=ALU.max, negate=True
                )

            def negm_half(h, hf):
                nq = NQB // 2
                ntp = apool.tile([nq, 128], fp32, name="ntp", tag="a")
                nc.tensor.transpose(
                    ntp[:, :], negp[h][:, nq * hf : nq * (hf + 1)], ident_sb[:, :]
                )
                nc.vector.tensor_copy(out=nT8[h][hf][:, :], in_=ntp[:, :])
                nc.sync.dma_start(
                    out=qt[h][17:18, 1024 * hf : 1024 * (hf + 1)].rearrange(
                        "a (b f) -> a b f", b=nq
                    ),
                    in_=nT8[h][hf][:, :],
                )

            # ---------------- pass B + ctx ----------------
            def b_iter(h, qh, kb, ctxc):
                st = stpool.tile([128, 1024], fp32, name="st", tag="st")
                lhs = kt[h][:, 128 * kb : 128 * (kb + 1)]
                for c in range(2):
                    nc.tensor.matmul(
                        st[:, 512 * c : 512 * (c + 1)],
                        lhsT=lhs,
                        rhs=qt[h][:, 1024 * qh + 512 * c : 1024 * qh + 512 * (c + 1)],
                        start=True,
                        stop=True,
                    )
                pt = ptpool.tile([128, 1024], f32r, name="pt", tag="pt")
                nc.scalar.activation(pt[:, :], st[:, :], AF.Exp)
                for c in range(2):
                    nc.tensor.matmul(
                        ctxc[c][0:17, :],
                        lhsT=vv[h][:, kb, :],
                        rhs=pt[:, 512 * c : 512 * (c + 1)],
                        start=(kb == 0),
                        stop=(kb == NKB - 1),
                    )

            def evac(h, qh, ctxc, last=False):
                eng = nc.vector if last else nc.scalar
                for c in range(2):
                    dst = ctxl[
                        32 * h : 32 * h + 17,
                        1024 * qh + 512 * c : 1024 * qh + 512 * (c + 1),
                    ]
                    if last:
                        nc.vector.tensor_copy(out=dst, in_=ctxc[c][0:17, :])
                    else:
                        nc.scalar.copy(dst, ctxc[c][0:17, :])

            def b_half(h, qh):
                return [
                    ctxpool.tile([17, 512], fp32, name=f"ctx{c}", tag="ctx")
                    for c in range(2)
                ]

            def finals(h, qh, ctxc=None):
                # l half-row -> [128,8] square, parallel reciprocal, back out
                q0 = 1024 * qh
                co = 16 * h + 8 * qh
                if False:
                    pass
                else:
                    nc.sync.dma_start(
                        out=lsq[:, co : co + 8],
                        in_=ctxl[32 * h + 16 : 32 * h + 17, q0 : q0 + 1024].rearrange(
                            "a (b f) -> a b f", b=128
                        ),
                    )
                nc.vector.reciprocal(lisq[:, co : co + 8], lsq[:, co : co + 8])
                nc.sync.dma_start(
                    out=ldram[h : h + 1, q0 : q0 + 1024].rearrange(
                        "a (b f) -> a b f", b=128
                    ),
                    in_=lisq[:, co : co + 8],
                )
                nc.sync.dma_start(
                    out=lbc[32 * h : 32 * h + 16, q0 : q0 + 1024],
                    in_=ldram[h : h + 1, q0 : q0 + 1024].to_broadcast([HD, 1024]),
                )
                nc.vector.tensor_tensor(
                    out=out_sb[32 * h : 32 * h + 16, q0 : q0 + 1024],
                    in0=ctxl[32 * h : 32 * h + 16, q0 : q0 + 1024],
                    in1=lbc[32 * h : 32 * h + 16, q0 : q0 + 1024],
                    op=ALU.mult,
                )
                nc.sync.dma_start(
                    out=out_d[16 * h : 16 * h + 16, q0 : q0 + 1024],
                    in_=out_sb[32 * h : 32 * h + 16, q0 : q0 + 1024],
                )

            # ---------------- schedule ----------------
            zero_fills()
            q_proj(0)
            k_proj(0, 512)
            # A(h0) first half interleaved with remaining projections (K first:
            # pass A and the kt dups depend on K, not on Q half 1)
            rest = [("k", o, min(512, NK - o)) for o in range(512, NK, 512)] + [
                ("q", 1, 0)
            ]
            for qb in range(NQB // 2):
                a_iter(0, qb)
                if rest:
                    kind, a1, a2 = rest.pop(0)
                    q_proj(a1) if kind == "q" else k_proj(a1, a2)
                if qb == 4:
                    k_dups()
            v_iter(0)
            v_iter(1)
            negm_half(0, 0)

            # B(h0,qh0) | A(h0) second half + V tail
            ctxc = b_half(0, 0)
            for kb in range(NKB):
                b_iter(0, 0, kb, ctxc)
                if kb < NQB // 2:
                    a_iter(0, NQB // 2 + kb)
                if kb + 2 < NKB:
                    v_iter(kb + 2)
            negm_half(0, 1)
            evac(0, 0, ctxc)

            # B(h0,qh1) | A(h1) first half
            ctxc = b_half(0, 1)
            for kb in range(NKB):
                b_iter(0, 1, kb, ctxc)
                if kb < NQB // 2:
                    a_iter(1, kb)
            negm_half(1, 0)
            evac(0, 1, ctxc)
            finals(0, 0)

            # B(h1,qh0) | A(h1) second half
            ctxc = b_half(1, 0)
            for kb in range(NKB):
                b_iter(1, 0, kb, ctxc)
                if kb < NQB // 2:
                    a_iter(1, NQB // 2 + kb)
            negm_half(1, 1)
            evac(1, 0, ctxc)
            finals(0, 1)

            # B(h1,qh1)
            ctxc = b_half(1, 1)
            for kb in range(NKB):
                b_iter(1, 1, kb, ctxc)
            finals(1, 0)
            evac(1, 1, ctxc, last=True)
            finals(1, 1)

    nc.finalize()
    return nc


def _prep_core_inputs(x, msk_add_full, w_query, w_key, w_value):
    """Build the 8 per-core input maps from full inputs.  Returns (maps, NKB)."""
    B = x.shape[0]
    onesrow = np.ones((1, S), dtype=np.float32)
    zrow = np.zeros((1, S), dtype=np.float32)
    identm = np.eye(E, dtype=np.float32)

    keeps = [np.flatnonzero(msk_add_full[b] == 0.0) for b in range(B)]
    max_nk = max(len(k) for k in keeps)
    assert max_nk >= NA, "row-max subsample needs >= NA valid keys"
    NKB = -(-max_nk // 128)  # ceil to 128
    NK = 128 * NKB

    per_batch = []
    for b in range(B):
        keep = keeps[b]
        nk = len(keep)
        xk = np.zeros((NK, E), dtype=np.float32)
        xk[:nk] = x[b][keep]
        maskrow = np.full((1, NK), NEG, dtype=np.float32)
        maskrow[0, :nk] = 0.0
        xTb = np.ascontiguousarray(x[b].T)
        xkTb = np.ascontiguousarray(xk.T)
        per_batch.append((xTb, xkTb, maskrow))

    in_maps = []
    for c in range(8):
        b = c // 4
        h0 = 2 * (c % 4)
        xTb, xkTb, maskrow = per_batch[b]

        def _pad48(w, scale=1.0):
            wc = np.zeros((E, 48), dtype=np.float32)
            wc[:, 0:16] = w[:, h0::8] * scale
            wc[:, 32:48] = w[:, h0 + 1 :: 8] * scale
            return wc

        in_maps.append(
            {
                "xT": xTb,
                "xkT": xkTb,
                "wq": _pad48(w_query, 0.25),  # 1/sqrt(hd) folded in (exact)
                "wk": _pad48(w_key),
                "wv": _pad48(w_value),
                "maskrow": maskrow,
                "onesrow": onesrow,
                "zrow": zrow,
                "ident": identm,
            }
        )
    return in_maps, NKB


def kernel(
    input_embeddings,
    token_attention_masks_source,
    token_attention_masks_target,
    masked,
    w_query,
    w_key,
    w_value,
):
    x = np.asarray(input_embeddings, dtype=np.float32)
    msk = np.asarray(token_attention_masks_source)
    wq_f = np.asarray(w_query, dtype=np.float32)
    wk_f = np.asarray(w_key, dtype=np.float32)
    wv_f = np.asarray(w_value, dtype=np.float32)
    assert int(np.asarray(masked)) == 0, "only the encoder (masked=0) path is supported"
    B = x.shape[0]
    assert x.shape == (2, S, E)

    msk_add = np.where(msk == 0, np.float32(NEG), np.float32(0.0))
    in_maps, NKB = _prep_core_inputs(x, msk_add, wq_f, wk_f, wv_f)

    if NKB not in _PROGS:
        _PROGS[NKB] = _build_program(NKB)
    nc = _PROGS[NKB]
    global _PROG
    _PROG = nc

    from concourse.bass_utils import run_bass_kernel_spmd

    res = run_bass_kernel_spmd(nc, in_maps, list(range(8)))

    out = np.empty((B, S, E), dtype=np.float32)
    for c in range(8):
        b = c // 4
        h0 = 2 * (c % 4)
        o = res.results[c]["out"]  # [32, 2048]
        out[b][:, h0::8] = o[0:16, :].T
        out[b][:, h0 + 1 :: 8] = o[16:32, :].T

    # The device row-max is a lower bound from a 512-key subsample; rows where
    # the true max exceeds it by >~88 overflow exp to inf (-> inf or NaN or,
    # when only the denominator overflows, an exact-zero vector).  Those rows
    # are deterministic and rare (<1%); recompute them exactly on host.
    for b in range(B):
        for h in range(8):
            hv = out[b][:, h::8]  # [S, 16]
            bad = ~np.isfinite(hv).all(axis=1) | (hv == 0.0).all(axis=1)
            if not bad.any():
                continue
            rows = np.flatnonzero(bad)
            xb = x[b].astype(np.float64)
            qh = (xb[rows] @ wq_f[:, h::8].astype(np.float64)) * 0.25
            kh = xb @ wk_f[:, h::8].astype(np.float64)
            vh = xb @ wv_f[:, h::8].astype(np.float64)
            sc = qh @ kh.T + msk_add[b][None, :].astype(np.float64)
            sc -= sc.max(axis=1, keepdims=True)
            p = np.exp(sc)
            p /= p.sum(axis=1, keepdims=True)
            out[b][rows, h::8] = (p @ vh).astype(np.float32)
    return out


_PROG = None


# revision 18
# speedup vs baseline: 1.2640x; 1.1080x over previous
"""Multi-head attention (B=2, H=8, S=2048, hd=16) on 8 Trainium2 NeuronCores.

Sharding: 16 (batch, head) groups -> 2 heads per core (cores 0-3: batch 0,
cores 4-7: batch 1).  Each core gets transposed embeddings, a key-compacted
copy (keys with source-mask 0 dropped; padded to NK with -1000 mask columns),
and the 32 projection-weight columns for its two heads.

Score matmuls run in float32r (1 cycle/row on the PE vs 4 for fp32) with
fp32-level accuracy recovered via split-precision row packing: K and Q are
each split into bf16-high + fp32-residual parts (Kh+Kl, Qh+Ql) and the four
cross products are packed into one 128-row contraction
  rows  0:16  Kh x Qh        rows 32:48  Kl x Qh(dup)
  rows 64:80  Kh(dup) x Ql   (Kl x Ql dropped: ~4e-4 score error)
  row 16: mask x ones        row 17: ones x (-rowmax)      (gaps zeroed)
Extra contraction rows are free (matmul cost is N output columns only), and
bf16-grid values pass through the PE's f32r truncation unchanged, so the sum
reconstructs the exact fp32 product.

Row-max comes from a cheap pass over the first NA=512 compacted keys (f32r,
bf16-grade): a lower-bound max keeps exp finite unless the excluded keys beat
the subsample by >88 (logistic tail, <1% of rows); those rows come back as
inf/NaN/zero and are recomputed exactly on the host.

ctx = P^T @ [V | 1] accumulates in PSUM with f32r operands; the ones column
gives the softmax denominator l; 1/l is computed on a [128,16] reshape (not
the serial [1,2048] row) and applied via DRAM-broadcast + DVE multiply.
Output per core is [32, 2048] (dim-major); the host scatters back into the
interleaved head layout.
"""

import numpy as np

S = 2048
E = 128
HD = 16
NQB = S // 128       # 16 query blocks
NEG = -1000.0
NA = 512             # keys sampled for the row-max pass

_PROGS = {}


def _build_program(NKB):
    import concourse.mybir as mybir
    from concourse import bacc
    from concourse.tile import TileContext

    NK = 128 * NKB

    fp32 = mybir.dt.float32
    f32r = mybir.dt.float32r
    bf16 = mybir.dt.bfloat16
    AF = mybir.ActivationFunctionType
    ALU = mybir.AluOpType
    AX = mybir.AxisListType

    nc = bacc.Bacc()

    xT = nc.declare_dram_parameter("xT", [E, S], fp32, isOutput=False)
    xkT = nc.declare_dram_parameter("xkT", [E, NK], fp32, isOutput=False)
    # weight columns padded to 48: head0 dims at 0:16, head1 dims at 32:48
    wq = nc.declare_dram_parameter("wq", [E, 48], fp32, isOutput=False)
    wk = nc.declare_dram_parameter("wk", [E, 48], fp32, isOutput=False)
    wv = nc.declare_dram_parameter("wv", [E, 48], fp32, isOutput=False)
    maskrow = nc.declare_dram_parameter("maskrow", [1, NK], f32r, isOutput=False)
    onesrow = nc.declare_dram_parameter("onesrow", [1, S], f32r, isOutput=False)
    zrow = nc.declare_dram_parameter("zrow", [1, S], f32r, isOutput=False)
    ident = nc.declare_dram_parameter("ident", [E, E], fp32, isOutput=False)
    out_d = nc.declare_dram_parameter("out", [2 * HD, S], fp32, isOutput=True)
    ldram = nc.dram_tensor("ldram", [2, S], fp32)

    with TileContext(nc) as tc:
        with (
            tc.tile_pool(name="consts", bufs=1) as cpool,
            tc.tile_pool(name="work", bufs=1) as wpool,
            tc.tile_pool(name="ptp", bufs=3) as ptpool,
            tc.tile_pool(name="stp", bufs=2, space="PSUM") as stpool,
            tc.tile_pool(name="ap", bufs=2, space="PSUM") as apool,
            tc.tile_pool(name="ctxp", bufs=2, space="PSUM") as ctxpool,
        ):
            # ---------------- input loads first (sync-queue order) ----------
            xT_sb = cpool.tile([E, S], fp32, name="xT_sb")
            wq_sb = cpool.tile([E, 48], fp32, name="wq_sb")
            wk_sb = cpool.tile([E, 48], fp32, name="wk_sb")
            wv_sb = cpool.tile([E, 48], fp32, name="wv_sb")
            xkT_sb = cpool.tile([E, NK], fp32, name="xkT_sb")
            ident_sb = cpool.tile([E, E], fp32, name="ident_sb")
            nc.sync.dma_start(out=xT_sb[:, 0:1024], in_=xT[:, 0:1024])
            nc.sync.dma_start(out=wq_sb[:, :], in_=wq[:, :])
            nc.sync.dma_start(out=xkT_sb[:, :], in_=xkT[:, :])
            nc.sync.dma_start(out=wk_sb[:, :], in_=wk[:, :])
            nc.sync.dma_start(out=xT_sb[:, 1024:2048], in_=xT[:, 1024:2048])
            nc.sync.dma_start(out=wv_sb[:, :], in_=wv[:, :])
            nc.sync.dma_start(out=ident_sb[:, :], in_=ident[:, :])

            # ---------------- persistent work tensors ----------------
            qt = [wpool.tile([128, S], f32r, name=f"qt{h}") for h in range(2)]
            kt = [wpool.tile([128, NK], f32r, name=f"kt{h}") for h in range(2)]
            qhb48 = wpool.tile([48, S], bf16, name="qhb48")
            khb48 = wpool.tile([48, NK], bf16, name="khb48")
            negid = wpool.tile([48, 48], bf16, name="negid")
            vv = [wpool.tile([128, NKB, HD + 1], f32r, name=f"vv{h}") for h in range(2)]
            negp = [wpool.tile([128, NQB], fp32, name=f"negp{h}") for h in range(2)]
            nT8 = [
                [wpool.tile([NQB // 2, 128], f32r, name=f"nT8_{h}{hf}") for hf in range(2)]
                for h in range(2)
            ]
            ctxl = wpool.tile([49, S], fp32, name="ctxl")
            lsq = wpool.tile([128, 2 * HD], fp32, name="lsq")
            lisq = wpool.tile([128, 2 * HD], fp32, name="lisq")
            lbc = wpool.tile([48, S], fp32, name="lbc")
            out_sb = wpool.tile([64, S], fp32, name="out_sb")

            # special rows (zero fills are issued later, after the first
            # projection loads, to keep the DMA queues free for the critical
            # input loads)
            for h in range(2):
                nc.sync.dma_start(out=qt[h][16:17, :], in_=onesrow[:, :])
                nc.sync.dma_start(out=kt[h][16:17, :], in_=maskrow[:, :])
                nc.sync.dma_start(out=kt[h][17:18, :], in_=onesrow[:, 0:NK])
                nc.sync.dma_start(
                    out=vv[h][:, :, HD : HD + 1],
                    in_=onesrow[0:1, 0:NKB].to_broadcast([128, NKB]),
                )

            nc.scalar.mul(negid[:, :], ident_sb[0:48, 0:48], -1.0)

            def zero_fills():
                # zero the gap rows (both sides: 0 * 0 avoids NaN from stale
                # SBUF).  Only true gaps are zeroed, so there are no WAW
                # dependencies against the split/dup writes and these DMAs can
                # be issued last.
                for h in range(2):
                    for lo, hi in ((18, 32), (48, 64)):
                        nc.sync.dma_start(
                            out=qt[h][lo:hi, :],
                            in_=zrow[0:1, 0:S].to_broadcast([hi - lo, S]),
                        )
                        nc.sync.dma_start(
                            out=kt[h][lo:hi, :],
                            in_=zrow[0:1, 0:NK].to_broadcast([hi - lo, NK]),
                        )

            # ---------------- projections + splits ----------------
            # Q: 1/sqrt(hd) folded into wq host-side.  Per 1024-col half:
            def q_proj(half):
                cs = slice(1024 * half, 1024 * (half + 1))
                qt_ps = stpool.tile([48, 1024], fp32, name="qt_ps", tag="st")
                for c in range(2):
                    nc.tensor.matmul(
                        qt_ps[:, 512 * c : 512 * (c + 1)],
                        lhsT=wq_sb[:, :],
                        rhs=xT_sb[:, 1024 * half + 512 * c : 1024 * half + 512 * (c + 1)],
                        start=True,
                        stop=False,
                    )
                nc.scalar.copy(qhb48[:, cs], qt_ps[:, :])        # bf16 round, both heads
                for c in range(2):                                    # PSUM -= Qh -> Ql
                    nc.tensor.matmul(
                        qt_ps[:, 512 * c : 512 * (c + 1)],
                        lhsT=negid[:, :],
                        rhs=qhb48[:, 1024 * half + 512 * c : 1024 * half + 512 * (c + 1)],
                        start=False,
                        stop=True,
                    )
                for h in range(2):
                    nc.vector.tensor_copy(
                        out=qt[h][0:16, cs], in_=qhb48[32 * h : 32 * h + 16, cs]
                    )
                    nc.vector.tensor_copy(
                        out=qt[h][64:80, cs], in_=qt_ps[32 * h : 32 * h + 16, :]
                    )
                    nc.sync.dma_start(out=qt[h][32:48, cs], in_=qt[h][0:16, cs])

            def k_proj(o, n):
                cs = slice(o, o + n)
                kt_ps = stpool.tile([48, 512], fp32, name="kt_ps", tag="st")
                nc.tensor.matmul(
                    kt_ps[:, 0:n], lhsT=wk_sb[:, :], rhs=xkT_sb[:, cs], start=True, stop=False
                )
                nc.scalar.copy(khb48[:, cs], kt_ps[:, 0:n])           # bf16 round, both heads
                nc.tensor.matmul(                                     # PSUM -= Kh -> Kl
                    kt_ps[:, 0:n], lhsT=negid[:, :], rhs=khb48[:, cs], start=False, stop=True
                )
                for h in range(2):
                    nc.scalar.copy(kt[h][0:16, cs], khb48[32 * h : 32 * h + 16, cs])
                    nc.scalar.copy(kt[h][32:48, cs], kt_ps[32 * h : 32 * h + 16, 0:n])

            def k_dups():
                for h in range(2):
                    nc.sync.dma_start(out=kt[h][64:80, :], in_=kt[h][0:16, :])

            def v_iter(kb):
                v_ps = apool.tile([128, 48], fp32, name="v_ps", tag="a")
                nc.tensor.matmul(
                    v_ps[:, :],
                    lhsT=xkT_sb[:, 128 * kb : 128 * (kb + 1)],
                    rhs=wv_sb[:, :],
                    start=True,
                    stop=True,
                )
                nc.vector.tensor_copy(out=vv[0][:, kb, 0:HD], in_=v_ps[:, 0:16])
                nc.vector.tensor_copy(out=vv[1][:, kb, 0:HD], in_=v_ps[:, 32:48])

            # ---------------- pass A: subsampled row-max ----------------
            def a_iter(h, qb):
                sc = apool.tile([128, NA], fp32, name="sc", tag="a")
                nc.tensor.matmul(
                    sc[:, :],
                    lhsT=qt[h][0:17, 128 * qb : 128 * (qb + 1)],
                    rhs=kt[h][0:17, 0:NA],
                    start=True,
                    stop=True,
                )
                nc.vector.tensor_reduce(
                    negp[h][:, qb : qb + 1], sc[:, :], axis=AX.X, op=ALU.max, negate=True
                )

            def negm_half(h, hf):
                nq = NQB // 2
                ntp = apool.tile([nq, 128], fp32, name="ntp", tag="a")
                nc.tensor.transpose(
                    ntp[:, :], negp[h][:, nq * hf : nq * (hf + 1)], ident_sb[:, :]
                )
                nc.vector.tensor_copy(out=nT8[h][hf][:, :], in_=ntp[:, :])
                nc.sync.dma_start(
                    out=qt[h][17:18, 1024 * hf : 1024 * (hf + 1)].rearrange(
                        "a (b f) -> a b f", b=nq
                    ),
                    in_=nT8[h][hf][:, :],
                )

            # ---------------- pass B + ctx ----------------
            def b_iter(h, qh, kb, ctxc):
                st = stpool.tile([128, 1024], fp32, name="st", tag="st")
                lhs = kt[h][0:80, 128 * kb : 128 * (kb + 1)]
                for c in range(2):
                    nc.tensor.matmul(
                        st[:, 512 * c : 512 * (c + 1)],
                        lhsT=lhs,
                        rhs=qt[h][0:80, 1024 * qh + 512 * c : 1024 * qh + 512 * (c + 1)],
                        start=True,
                        stop=True,
                    )
                pt = ptpool.tile([128, 1024], f32r, name="pt", tag="pt")
                nc.scalar.activation(pt[:, :], st[:, :], AF.Exp)
                for c in range(2):
                    nc.tensor.matmul(
                        ctxc[c][0:17, :],
                        lhsT=vv[h][:, kb, :],
                        rhs=pt[:, 512 * c : 512 * (c + 1)],
                        start=(kb == 0),
                        stop=(kb == NKB - 1),
                    )

            def evac(h, qh, ctxc, last=False):
                eng = nc.vector if last else nc.scalar
                for c in range(2):
                    dst = ctxl[
                        32 * h : 32 * h + 17,
                        1024 * qh + 512 * c : 1024 * qh + 512 * (c + 1),
                    ]
                    if last:
                        nc.vector.tensor_copy(out=dst, in_=ctxc[c][0:17, :])
                    else:
                        nc.scalar.copy(dst, ctxc[c][0:17, :])

            def b_half(h, qh):
                return [
                    ctxpool.tile([17, 512], fp32, name=f"ctx{c}", tag="ctx")
                    for c in range(2)
                ]

            def finals(h, qh, ctxc=None):
                # l half-row -> [128,8] square, parallel reciprocal, back out
                q0 = 1024 * qh
                co = 16 * h + 8 * qh
                if False:
                    pass
                else:
                    nc.sync.dma_start(
                        out=lsq[:, co : co + 8],
                        in_=ctxl[32 * h + 16 : 32 * h + 17, q0 : q0 + 1024].rearrange(
                            "a (b f) -> a b f", b=128
                        ),
                    )
                nc.vector.reciprocal(lisq[:, co : co + 8], lsq[:, co : co + 8])
                nc.sync.dma_start(
                    out=ldram[h : h + 1, q0 : q0 + 1024].rearrange(
                        "a (b f) -> a b f", b=128
                    ),
                    in_=lisq[:, co : co + 8],
                )
                nc.sync.dma_start(
                    out=lbc[32 * h : 32 * h + 16, q0 : q0 + 1024],
                    in_=ldram[h : h + 1, q0 : q0 + 1024].to_broadcast([HD, 1024]),
                )
                nc.vector.tensor_tensor(
                    out=out_sb[32 * h : 32 * h + 16, q0 : q0 + 1024],
                    in0=ctxl[32 * h : 32 * h + 16, q0 : q0 + 1024],
                    in1=lbc[32 * h : 32 * h + 16, q0 : q0 + 1024],
                    op=ALU.mult,
                )
                nc.sync.dma_start(
                    out=out_d[16 * h : 16 * h + 16, q0 : q0 + 1024],
                    in_=out_sb[32 * h : 32 * h + 16, q0 : q0 + 1024],
                )

            # ---------------- schedule ----------------
            q_proj(0)
            k_proj(0, 512)
            # A(h0) first half interleaved with remaining projections (K first:
            # pass A and the kt dups depend on K, not on Q half 1)
            rest = [("k", o, min(512, NK - o)) for o in range(512, NK, 512)] + [
                ("q", 1, 0)
            ]
            for qb in range(NQB // 2):
                a_iter(0, qb)
                if rest:
                    kind, a1, a2 = rest.pop(0)
                    q_proj(a1) if kind == "q" else k_proj(a1, a2)
                if qb == 4:
                    k_dups()
            zero_fills()
            v_iter(0)
            v_iter(1)
            negm_half(0, 0)

            # B(h0,qh0) | A(h0) second half + V tail
            ctxc = b_half(0, 0)
            for kb in range(NKB):
                b_iter(0, 0, kb, ctxc)
                if kb < NQB // 2:
                    a_iter(0, NQB // 2 + kb)
                if kb + 2 < NKB:
                    v_iter(kb + 2)
            negm_half(0, 1)
            evac(0, 0, ctxc)

            # B(h0,qh1) | A(h1) first half
            ctxc = b_half(0, 1)
            for kb in range(NKB):
                b_iter(0, 1, kb, ctxc)
                if kb < NQB // 2:
                    a_iter(1, kb)
            negm_half(1, 0)
            evac(0, 1, ctxc)
            finals(0, 0)

            # B(h1,qh0) | A(h1) second half
            ctxc = b_half(1, 0)
            for kb in range(NKB):
                b_iter(1, 0, kb, ctxc)
                if kb < NQB // 2:
                    a_iter(1, NQB // 2 + kb)
            negm_half(1, 1)
            evac(1, 0, ctxc)
            finals(0, 1)

            # B(h1,qh1)
            ctxc = b_half(1, 1)
            for kb in range(NKB):
                b_iter(1, 1, kb, ctxc)
            finals(1, 0)
            evac(1, 1, ctxc, last=True)
            finals(1, 1)

    nc.finalize()
    return nc


def _prep_core_inputs(x, msk_add_full, w_query, w_key, w_value):
    """Build the 8 per-core input maps from full inputs.  Returns (maps, NKB)."""
    B = x.shape[0]
    onesrow = np.ones((1, S), dtype=np.float32)
    zrow = np.zeros((1, S), dtype=np.float32)
    identm = np.eye(E, dtype=np.float32)

    keeps = [np.flatnonzero(msk_add_full[b] == 0.0) for b in range(B)]
    max_nk = max(len(k) for k in keeps)
    assert max_nk >= NA, "row-max subsample needs >= NA valid keys"
    NKB = -(-max_nk // 128)  # ceil to 128
    NK = 128 * NKB

    per_batch = []
    for b in range(B):
        keep = keeps[b]
        nk = len(keep)
        xk = np.zeros((NK, E), dtype=np.float32)
        xk[:nk] = x[b][keep]
        maskrow = np.full((1, NK), NEG, dtype=np.float32)
        maskrow[0, :nk] = 0.0
        xTb = np.ascontiguousarray(x[b].T)
        xkTb = np.ascontiguousarray(xk.T)
        per_batch.append((xTb, xkTb, maskrow))

    in_maps = []
    for c in range(8):
        b = c // 4
        h0 = 2 * (c % 4)
        xTb, xkTb, maskrow = per_batch[b]

        def _pad48(w, scale=1.0):
            wc = np.zeros((E, 48), dtype=np.float32)
            wc[:, 0:16] = w[:, h0::8] * scale
            wc[:, 32:48] = w[:, h0 + 1 :: 8] * scale
            return wc

        in_maps.append(
            {
                "xT": xTb,
                "xkT": xkTb,
                "wq": _pad48(w_query, 0.25),  # 1/sqrt(hd) folded in (exact)
                "wk": _pad48(w_key),
                "wv": _pad48(w_value),
                "maskrow": maskrow,
                "onesrow": onesrow,
                "zrow": zrow,
                "ident": identm,
            }
        )
    return in_maps, NKB


def kernel(
    input_embeddings,
    token_attention_masks_source,
    token_attention_masks_target,
    masked,
    w_query,
    w_key,
    w_value,
):
    x = np.asarray(input_embeddings, dtype=np.float32)
    msk = np.asarray(token_attention_masks_source)
    wq_f = np.asarray(w_query, dtype=np.float32)
    wk_f = np.asarray(w_key, dtype=np.float32)
    wv_f = np.asarray(w_value, dtype=np.float32)
    assert int(np.asarray(masked)) == 0, "only the encoder (masked=0) path is supported"
    B = x.shape[0]
    assert x.shape == (2, S, E)

    msk_add = np.where(msk == 0, np.float32(NEG), np.float32(0.0))
    in_maps, NKB = _prep_core_inputs(x, msk_add, wq_f, wk_f, wv_f)

    if NKB not in _PROGS:
        _PROGS[NKB] = _build_program(NKB)
    nc = _PROGS[NKB]
    global _PROG
    _PROG = nc

    from concourse.bass_utils import run_bass_kernel_spmd

    res = run_bass_kernel_spmd(nc, in_maps, list(range(8)))

    out = np.empty((B, S, E), dtype=np.float32)
    for c in range(8):
        b = c // 4
        h0 = 2 * (c % 4)
        o = res.results[c]["out"]  # [32, 2048]
        out[b][:, h0::8] = o[0:16, :].T
        out[b][:, h0 + 1 :: 8] = o[16:32, :].T

    # The device row-max is a lower bound from a 512-key subsample; rows where
    # the true max exceeds it by >~88 overflow exp to inf (-> inf or NaN or,
    # when only the denominator overflows, an exact-zero vector).  Those rows
    # are deterministic and rare (<1%); recompute them exactly on host.
    for b in range(B):
        for h in range(8):
            hv = out[b][:, h::8]  # [S, 16]
            bad = ~np.isfinite(hv).all(axis=1) | (hv == 0.0).all(axis=1)
            if not bad.any():
                continue
            rows = np.flatnonzero(bad)
            xb = x[b].astype(np.float64)
            qh = (xb[rows] @ wq_f[:, h::8].astype(np.float64)) * 0.25
            kh = xb @ wk_f[:, h::8].astype(np.float64)
            vh = xb @ wv_f[:, h::8].astype(np.float64)
            sc = qh @ kh.T + msk_add[b][None, :].astype(np.float64)
            sc -= sc.max(axis=1, keepdims=True)
            p = np.exp(sc)
            p /= p.sum(axis=1, keepdims=True)
            out[b][rows, h::8] = (p @ vh).astype(np.float32)
    return out


_PROG = None


# revision 20
# speedup vs baseline: 1.3414x; 1.0612x over previous
"""Multi-head attention (B=2, H=8, S=2048, hd=16) on 8 Trainium2 NeuronCores.

Sharding: 16 (batch, head) groups -> 2 heads per core (cores 0-3: batch 0,
cores 4-7: batch 1).  Each core gets transposed embeddings, a key-compacted
copy (keys with source-mask 0 dropped; padded to NK with -1000 mask columns),
and the 32 projection-weight columns for its two heads.

Score matmuls run in float32r (1 cycle/row on the PE vs 4 for fp32) with
fp32-level accuracy recovered via split-precision row packing: K and Q are
each split into bf16-high + fp32-residual parts (Kh+Kl, Qh+Ql) and the four
cross products are packed into one 128-row contraction
  rows  0:16  Kh x Qh        rows 32:48  Kl x Qh(dup)
  rows 64:80  Kh(dup) x Ql   (Kl x Ql dropped: ~4e-4 score error)
  row 16: mask x ones        row 17: ones x (-rowmax)      (gaps zeroed)
Extra contraction rows are free (matmul cost is N output columns only), and
bf16-grid values pass through the PE's f32r truncation unchanged, so the sum
reconstructs the exact fp32 product.

Row-max comes from a cheap pass over the first NA=512 compacted keys (f32r,
bf16-grade): a lower-bound max keeps exp finite unless the excluded keys beat
the subsample by >88 (logistic tail, <1% of rows); those rows come back as
inf/NaN/zero and are recomputed exactly on the host.

ctx = P^T @ [V | 1] accumulates in PSUM with f32r operands; the ones column
gives the softmax denominator l; 1/l is computed on a [128,16] reshape (not
the serial [1,2048] row) and applied via DRAM-broadcast + DVE multiply.
Output per core is [32, 2048] (dim-major); the host scatters back into the
interleaved head layout.
"""

import numpy as np

S = 2048
E = 128
HD = 16
NQB = S // 128       # 16 query blocks
NEG = -1000.0
NA = 512             # keys sampled for the row-max pass

_PROGS = {}


def _build_program(NKB):
    import concourse.mybir as mybir
    from concourse import bacc
    from concourse.tile import TileContext

    NK = 128 * NKB

    fp32 = mybir.dt.float32
    f32r = mybir.dt.float32r
    bf16 = mybir.dt.bfloat16
    AF = mybir.ActivationFunctionType
    ALU = mybir.AluOpType
    AX = mybir.AxisListType

    nc = bacc.Bacc()

    xT = nc.declare_dram_parameter("xT", [E, S], fp32, isOutput=False)
    xkT = nc.declare_dram_parameter("xkT", [E, NK], fp32, isOutput=False)
    # weight columns padded to 48: head0 dims at 0:16, head1 dims at 32:48
    wq = nc.declare_dram_parameter("wq", [E, 48], fp32, isOutput=False)
    wk = nc.declare_dram_parameter("wk", [E, 48], fp32, isOutput=False)
    wv = nc.declare_dram_parameter("wv", [E, 48], fp32, isOutput=False)
    maskrow = nc.declare_dram_parameter("maskrow", [1, NK], f32r, isOutput=False)
    onesrow = nc.declare_dram_parameter("onesrow", [1, S], f32r, isOutput=False)
    zrow = nc.declare_dram_parameter("zrow", [1, S], f32r, isOutput=False)
    ident = nc.declare_dram_parameter("ident", [E, E], fp32, isOutput=False)
    out_d = nc.declare_dram_parameter("out", [2 * HD, S], fp32, isOutput=True)
    ldram = nc.dram_tensor("ldram", [2, S], fp32)

    with TileContext(nc) as tc:
        with (
            tc.tile_pool(name="consts", bufs=1) as cpool,
            tc.tile_pool(name="work", bufs=1) as wpool,
            tc.tile_pool(name="ptp", bufs=3) as ptpool,
            tc.tile_pool(name="stp", bufs=2, space="PSUM") as stpool,
            tc.tile_pool(name="ap", bufs=2, space="PSUM") as apool,
            tc.tile_pool(name="ctxp", bufs=2, space="PSUM") as ctxpool,
        ):
            # ---------------- input loads first (sync-queue order) ----------
            xT_sb = cpool.tile([E, S], fp32, name="xT_sb")
            wq_sb = cpool.tile([E, 48], fp32, name="wq_sb")
            wk_sb = cpool.tile([E, 48], fp32, name="wk_sb")
            wv_sb = cpool.tile([E, 48], fp32, name="wv_sb")
            xkT_sb = cpool.tile([E, NK], fp32, name="xkT_sb")
            ident_sb = cpool.tile([E, E], fp32, name="ident_sb")
            nc.sync.dma_start(out=xT_sb[:, 0:512], in_=xT[:, 0:512])
            nc.sync.dma_start(out=xT_sb[:, 512:1024], in_=xT[:, 512:1024])
            nc.sync.dma_start(out=wq_sb[:, :], in_=wq[:, :])
            for o in range(0, NK, 512):
                n = min(512, NK - o)
                nc.sync.dma_start(out=xkT_sb[:, o : o + n], in_=xkT[:, o : o + n])
            nc.sync.dma_start(out=wk_sb[:, :], in_=wk[:, :])
            nc.sync.dma_start(out=xT_sb[:, 1024:1536], in_=xT[:, 1024:1536])
            nc.sync.dma_start(out=xT_sb[:, 1536:2048], in_=xT[:, 1536:2048])
            nc.sync.dma_start(out=wv_sb[:, :], in_=wv[:, :])
            nc.sync.dma_start(out=ident_sb[:, :], in_=ident[:, :])

            # ---------------- persistent work tensors ----------------
            qt = [wpool.tile([128, S], f32r, name=f"qt{h}") for h in range(2)]
            kt = [wpool.tile([128, NK], f32r, name=f"kt{h}") for h in range(2)]
            qhb48 = wpool.tile([48, S], bf16, name="qhb48")
            khb48 = wpool.tile([48, NK], bf16, name="khb48")
            negid = wpool.tile([48, 48], bf16, name="negid")
            vv = [wpool.tile([128, NKB, HD + 1], f32r, name=f"vv{h}") for h in range(2)]
            negp = [wpool.tile([128, NQB], fp32, name=f"negp{h}") for h in range(2)]
            nT8 = [
                [wpool.tile([NQB // 2, 128], f32r, name=f"nT8_{h}{hf}") for hf in range(2)]
                for h in range(2)
            ]
            ctxl = wpool.tile([49, S], fp32, name="ctxl")
            lsq = wpool.tile([128, 2 * HD], fp32, name="lsq")
            lisq = wpool.tile([128, 2 * HD], fp32, name="lisq")
            lbc = wpool.tile([48, S], fp32, name="lbc")
            out_sb = wpool.tile([64, S], fp32, name="out_sb")

            # special rows (zero fills are issued later, after the first
            # projection loads, to keep the DMA queues free for the critical
            # input loads)
            for h in range(2):
                nc.sync.dma_start(out=qt[h][16:17, :], in_=onesrow[:, :])
                nc.sync.dma_start(out=kt[h][16:17, :], in_=maskrow[:, :])
                nc.sync.dma_start(out=kt[h][17:18, :], in_=onesrow[:, 0:NK])
                nc.sync.dma_start(
                    out=vv[h][:, :, HD : HD + 1],
                    in_=onesrow[0:1, 0:NKB].to_broadcast([128, NKB]),
                )

            nc.scalar.mul(negid[:, :], ident_sb[0:48, 0:48], -1.0)

            def zero_fills():
                # zero the gap rows (both sides: 0 * 0 avoids NaN from stale
                # SBUF).  Only true gaps are zeroed, so there are no WAW
                # dependencies against the split/dup writes and these DMAs can
                # be issued last.
                for h in range(2):
                    for lo, hi in ((18, 32), (48, 64)):
                        nc.sync.dma_start(
                            out=qt[h][lo:hi, :],
                            in_=zrow[0:1, 0:S].to_broadcast([hi - lo, S]),
                        )
                        nc.sync.dma_start(
                            out=kt[h][lo:hi, :],
                            in_=zrow[0:1, 0:NK].to_broadcast([hi - lo, NK]),
                        )

            # ---------------- projections + splits ----------------
            # Q: 1/sqrt(hd) folded into wq host-side.  Per 1024-col half:
            def q_proj(half):
                cs = slice(1024 * half, 1024 * (half + 1))
                qt_ps = stpool.tile([48, 1024], fp32, name="qt_ps", tag="st")
                for c in range(2):
                    nc.tensor.matmul(
                        qt_ps[:, 512 * c : 512 * (c + 1)],
                        lhsT=wq_sb[:, :],
                        rhs=xT_sb[:, 1024 * half + 512 * c : 1024 * half + 512 * (c + 1)],
                        start=True,
                        stop=False,
                    )
                nc.scalar.copy(qhb48[:, cs], qt_ps[:, :])        # bf16 round, both heads
                for c in range(2):                                    # PSUM -= Qh -> Ql
                    nc.tensor.matmul(
                        qt_ps[:, 512 * c : 512 * (c + 1)],
                        lhsT=negid[:, :],
                        rhs=qhb48[:, 1024 * half + 512 * c : 1024 * half + 512 * (c + 1)],
                        start=False,
                        stop=True,
                    )
                for h in range(2):
                    nc.vector.tensor_copy(
                        out=qt[h][0:16, cs], in_=qhb48[32 * h : 32 * h + 16, cs]
                    )
                    nc.vector.tensor_copy(
                        out=qt[h][64:80, cs], in_=qt_ps[32 * h : 32 * h + 16, :]
                    )
                    nc.sync.dma_start(out=qt[h][32:48, cs], in_=qt[h][0:16, cs])

            def k_proj(o, n):
                cs = slice(o, o + n)
                kt_ps = stpool.tile([48, 512], fp32, name="kt_ps", tag="st")
                nc.tensor.matmul(
                    kt_ps[:, 0:n], lhsT=wk_sb[:, :], rhs=xkT_sb[:, cs], start=True, stop=False
                )
                nc.scalar.copy(khb48[:, cs], kt_ps[:, 0:n])           # bf16 round, both heads
                nc.tensor.matmul(                                     # PSUM -= Kh -> Kl
                    kt_ps[:, 0:n], lhsT=negid[:, :], rhs=khb48[:, cs], start=False, stop=True
                )
                for h in range(2):
                    nc.scalar.copy(kt[h][0:16, cs], khb48[32 * h : 32 * h + 16, cs])
                    nc.scalar.copy(kt[h][32:48, cs], kt_ps[32 * h : 32 * h + 16, 0:n])

            def k_dups():
                for h in range(2):
                    nc.sync.dma_start(out=kt[h][64:80, :], in_=kt[h][0:16, :])

            def v_iter(kb):
                v_ps = apool.tile([128, 48], fp32, name="v_ps", tag="a")
                nc.tensor.matmul(
                    v_ps[:, :],
                    lhsT=xkT_sb[:, 128 * kb : 128 * (kb + 1)],
                    rhs=wv_sb[:, :],
                    start=True,
                    stop=True,
                )
                nc.vector.tensor_copy(out=vv[0][:, kb, 0:HD], in_=v_ps[:, 0:16])
                nc.vector.tensor_copy(out=vv[1][:, kb, 0:HD], in_=v_ps[:, 32:48])

            # ---------------- pass A: subsampled row-max ----------------
            def a_iter(h, qb):
                sc = apool.tile([128, NA], fp32, name="sc", tag="a")
                nc.tensor.matmul(
                    sc[:, :],
                    lhsT=qt[h][0:17, 128 * qb : 128 * (qb + 1)],
                    rhs=kt[h][0:17, 0:NA],
                    start=True,
                    stop=True,
                )
                nc.vector.tensor_reduce(
                    negp[h][:, qb : qb + 1], sc[:, :], axis=AX.X, op=ALU.max, negate=True
                )

            def negm_half(h, hf):
                nq = NQB // 2
                ntp = apool.tile([nq, 128], fp32, name="ntp", tag="a")
                nc.tensor.transpose(
                    ntp[:, :], negp[h][:, nq * hf : nq * (hf + 1)], ident_sb[:, :]
                )
                nc.vector.tensor_copy(out=nT8[h][hf][:, :], in_=ntp[:, :])
                nc.sync.dma_start(
                    out=qt[h][17:18, 1024 * hf : 1024 * (hf + 1)].rearrange(
                        "a (b f) -> a b f", b=nq
                    ),
                    in_=nT8[h][hf][:, :],
                )

            # ---------------- pass B + ctx (software-pipelined) --------
            def st_exp(h, qh, kb):
                st = stpool.tile([128, 1024], fp32, name="st", tag="st")
                lhs = kt[h][0:80, 128 * kb : 128 * (kb + 1)]
                for c in range(2):
                    nc.tensor.matmul(
                        st[:, 512 * c : 512 * (c + 1)],
                        lhsT=lhs,
                        rhs=qt[h][0:80, 1024 * qh + 512 * c : 1024 * qh + 512 * (c + 1)],
                        start=True,
                        stop=True,
                    )
                pt = ptpool.tile([128, 1024], f32r, name="pt", tag="pt")
                nc.scalar.activation(pt[:, :], st[:, :], AF.Exp)
                return pt

            def ctx_mm(h, kb, ctxc, pt):
                for c in range(2):
                    nc.tensor.matmul(
                        ctxc[c][0:17, :],
                        lhsT=vv[h][:, kb, :],
                        rhs=pt[:, 512 * c : 512 * (c + 1)],
                        start=(kb == 0),
                        stop=(kb == NKB - 1),
                    )

            def evac(h, qh, ctxc):
                for c in range(2):
                    nc.vector.tensor_copy(
                        out=ctxl[
                            32 * h : 32 * h + 17,
                            1024 * qh + 512 * c : 1024 * qh + 512 * (c + 1),
                        ],
                        in_=ctxc[c][0:17, :],
                    )

            def b_half(h, qh):
                return [
                    ctxpool.tile([17, 512], fp32, name=f"ctx{c}", tag="ctx")
                    for c in range(2)
                ]

            def finals(h, qh, ctxc=None):
                # l half-row -> [128,8] square, parallel reciprocal, back out
                q0 = 1024 * qh
                co = 16 * h + 8 * qh
                if False:
                    pass
                else:
                    nc.sync.dma_start(
                        out=lsq[:, co : co + 8],
                        in_=ctxl[32 * h + 16 : 32 * h + 17, q0 : q0 + 1024].rearrange(
                            "a (b f) -> a b f", b=128
                        ),
                    )
                nc.vector.reciprocal(lisq[:, co : co + 8], lsq[:, co : co + 8])
                nc.sync.dma_start(
                    out=ldram[h : h + 1, q0 : q0 + 1024].rearrange(
                        "a (b f) -> a b f", b=128
                    ),
                    in_=lisq[:, co : co + 8],
                )
                nc.sync.dma_start(
                    out=lbc[32 * h : 32 * h + 16, q0 : q0 + 1024],
                    in_=ldram[h : h + 1, q0 : q0 + 1024].to_broadcast([HD, 1024]),
                )
                nc.vector.tensor_tensor(
                    out=out_sb[32 * h : 32 * h + 16, q0 : q0 + 1024],
                    in0=ctxl[32 * h : 32 * h + 16, q0 : q0 + 1024],
                    in1=lbc[32 * h : 32 * h + 16, q0 : q0 + 1024],
                    op=ALU.mult,
                )
                nc.sync.dma_start(
                    out=out_d[16 * h : 16 * h + 16, q0 : q0 + 1024],
                    in_=out_sb[32 * h : 32 * h + 16, q0 : q0 + 1024],
                )

            # ---------------- schedule ----------------
            q_proj(0)
            k_proj(0, 512)
            # A(h0) first half interleaved with remaining projections (K first:
            # pass A and the kt dups depend on K, not on Q half 1)
            rest = [("k", o, min(512, NK - o)) for o in range(512, NK, 512)] + [
                ("q", 1, 0)
            ]
            for qb in range(NQB // 2):
                a_iter(0, qb)
                if rest:
                    kind, a1, a2 = rest.pop(0)
                    q_proj(a1) if kind == "q" else k_proj(a1, a2)
                if qb == 4:
                    k_dups()
                if qb >= 5:
                    v_iter(qb - 5)
            zero_fills()
            for kb in range(3, NKB):
                v_iter(kb)
            negm_half(0, 0)

            def b_stream(h, qh, inter):
                ctxc = b_half(h, qh)
                prev = None
                for kb in range(NKB):
                    pt = st_exp(h, qh, kb)
                    if prev is not None:
                        ctx_mm(h, kb - 1, ctxc, prev)
                    prev = pt
                    if inter:
                        inter.pop(0)()
                ctx_mm(h, NKB - 1, ctxc, prev)
                while inter:
                    inter.pop(0)()
                return ctxc

            # B(h0,qh0) | A(h0) second half
            ctxc = b_stream(0, 0, [
                (lambda qb=q: a_iter(0, NQB // 2 + qb)) for q in range(NQB // 2)
            ])
            negm_half(0, 1)
            evac(0, 0, ctxc)

            # B(h0,qh1) | A(h1) first half
            ctxc = b_stream(0, 1, [
                (lambda qb=q: a_iter(1, qb)) for q in range(NQB // 2)
            ])
            negm_half(1, 0)
            evac(0, 1, ctxc)
            finals(0, 0)

            # B(h1,qh0) | A(h1) second half
            ctxc = b_stream(1, 0, [
                (lambda qb=q: a_iter(1, NQB // 2 + qb)) for q in range(NQB // 2)
            ])
            negm_half(1, 1)
            evac(1, 0, ctxc)
            finals(0, 1)

            # B(h1,qh1)
            ctxc = b_stream(1, 1, [])
            finals(1, 0)
            evac(1, 1, ctxc)
            finals(1, 1)

    nc.finalize()
    return nc


def _prep_core_inputs(x, msk_add_full, w_query, w_key, w_value):
    """Build the 8 per-core input maps from full inputs.  Returns (maps, NKB)."""
    B = x.shape[0]
    onesrow = np.ones((1, S), dtype=np.float32)
    zrow = np.zeros((1, S), dtype=np.float32)
    identm = np.eye(E, dtype=np.float32)

    keeps = [np.flatnonzero(msk_add_full[b] == 0.0) for b in range(B)]
    max_nk = max(len(k) for k in keeps)
    assert max_nk >= NA, "row-max subsample needs >= NA valid keys"
    NKB = -(-max_nk // 128)  # ceil to 128
    NK = 128 * NKB

    per_batch = []
    for b in range(B):
        keep = keeps[b]
        nk = len(keep)
        xk = np.zeros((NK, E), dtype=np.float32)
        xk[:nk] = x[b][keep]
        maskrow = np.full((1, NK), NEG, dtype=np.float32)
        maskrow[0, :nk] = 0.0
        xTb = np.ascontiguousarray(x[b].T)
        xkTb = np.ascontiguousarray(xk.T)
        per_batch.append((xTb, xkTb, maskrow))

    in_maps = []
    for c in range(8):
        b = c // 4
        h0 = 2 * (c % 4)
        xTb, xkTb, maskrow = per_batch[b]

        def _pad48(w, scale=1.0):
            wc = np.zeros((E, 48), dtype=np.float32)
            wc[:, 0:16] = w[:, h0::8] * scale
            wc[:, 32:48] = w[:, h0 + 1 :: 8] * scale
            return wc

        in_maps.append(
            {
                "xT": xTb,
                "xkT": xkTb,
                "wq": _pad48(w_query, 0.25),  # 1/sqrt(hd) folded in (exact)
                "wk": _pad48(w_key),
                "wv": _pad48(w_value),
                "maskrow": maskrow,
                "onesrow": onesrow,
                "zrow": zrow,
                "ident": identm,
            }
        )
    return in_maps, NKB


def kernel(
    input_embeddings,
    token_attention_masks_source,
    token_attention_masks_target,
    masked,
    w_query,
    w_key,
    w_value,
):
    x = np.asarray(input_embeddings, dtype=np.float32)
    msk = np.asarray(token_attention_masks_source)
    wq_f = np.asarray(w_query, dtype=np.float32)
    wk_f = np.asarray(w_key, dtype=np.float32)
    wv_f = np.asarray(w_value, dtype=np.float32)
    assert int(np.asarray(masked)) == 0, "only the encoder (masked=0) path is supported"
    B = x.shape[0]
    assert x.shape == (2, S, E)

    msk_add = np.where(msk == 0, np.float32(NEG), np.float32(0.0))
    in_maps, NKB = _prep_core_inputs(x, msk_add, wq_f, wk_f, wv_f)

    if NKB not in _PROGS:
        _PROGS[NKB] = _build_program(NKB)
    nc = _PROGS[NKB]
    global _PROG
    _PROG = nc

    from concourse.bass_utils import run_bass_kernel_spmd

    res = run_bass_kernel_spmd(nc, in_maps, list(range(8)))

    out = np.empty((B, S, E), dtype=np.float32)
    for c in range(8):
        b = c // 4
        h0 = 2 * (c % 4)
        o = res.results[c]["out"]  # [32, 2048]
        out[b][:, h0::8] = o[0:16, :].T
        out[b][:, h0 + 1 :: 8] = o[16:32, :].T

    # The device row-max is a lower bound from a 512-key subsample; rows where
    # the true max exceeds it by >~88 overflow exp to inf (-> inf or NaN or,
    # when only the denominator overflows, an exact-zero vector).  Those rows
    # are deterministic and rare (<1%); recompute them exactly on host.
    for b in range(B):
        for h in range(8):
            hv = out[b][:, h::8]  # [S, 16]
            bad = ~np.isfinite(hv).all(axis=1) | (hv == 0.0).all(axis=1)
            if not bad.any():
                continue
            rows = np.flatnonzero(bad)
            xb = x[b].astype(np.float64)
            qh = (xb[rows] @ wq_f[:, h::8].astype(np.float64)) * 0.25
            kh = xb @ wk_f[:, h::8].astype(np.float64)
            vh = xb @ wv_f[:, h::8].astype(np.float64)
            sc = qh @ kh.T + msk_add[b][None, :].astype(np.float64)
            sc -= sc.max(axis=1, keepdims=True)
            p = np.exp(sc)
            p /= p.sum(axis=1, keepdims=True)
            out[b][rows, h::8] = (p @ vh).astype(np.float32)
    return out


_PROG = None
